# revision 10
# baseline (speedup 1.0000x reference)
"""Causal self-attention kernel for 8 trn2 NeuronCores.

Sharding: core c = 2*b + g handles batch b (of 4) and head-group g (of 2,
8 heads each).  Each core computes QKV projection, causal attention and the
partial output projection for its head-group; the host sums the two
head-group partials per batch (the w_proj row-split all-reduce done on host).

Single fused schedule: the attention jd-loop for query panel Q is
software-pipelined with the QKV projection of t-panel Q+1 and the output
projection of panel Q-1, so the PE never idles while the scalar engine
computes exp.  Causal structure is exploited by N-trimming the QK matmuls
and exp activations on the diagonal key blocks (only a single 128x128
triangle mask remains), and the softmax denominator comes free from a
ones-column appended to V (M=65 PV matmul).
"""

import sys

if "/opt/trn_rl_repo" not in sys.path:
    sys.path.insert(0, "/opt/trn_rl_repo")

from collections import deque
from contextlib import ExitStack

import numpy as np

import concourse.bass as bass
import concourse.mybir as mybir
import concourse.tile as tile
from concourse import bacc
from concourse.bass_utils import run_bass_kernel_spmd
from concourse.masks import make_identity

F32 = mybir.dt.float32
BF16 = mybir.dt.bfloat16
AF = mybir.ActivationFunctionType

B, T, C = 4, 2048, 1024
N_HEAD = 16
HEAD_DIM = 64
N_CORES = 8
HPC = 8          # heads per core
GC = 512         # head-group channel width (8 heads * 64)
SCALE = 0.125    # 1/sqrt(64)

T_PANEL = 512
NP = T // T_PANEL    # 4 panels (shared for t and q)


def build_program():
    nc = bacc.Bacc(
        "TRN2", target_bir_lowering=False, debug=False, num_devices=N_CORES
    )
    x_ap = nc.dram_tensor("x", [T, C], F32, kind="ExternalInput").ap()
    wq_ap = nc.dram_tensor("wq", [C, GC], F32, kind="ExternalInput").ap()
    wk_ap = nc.dram_tensor("wk", [C, GC], F32, kind="ExternalInput").ap()
    wv_ap = nc.dram_tensor("wv", [C, GC], F32, kind="ExternalInput").ap()
    wp_ap = nc.dram_tensor("wp", [GC, C], F32, kind="ExternalInput").ap()
    out_ap = nc.dram_tensor("out", [T, C], F32, kind="ExternalOutput").ap()

    with ExitStack() as ctx:
        tc = ctx.enter_context(tile.TileContext(nc))
        build_kernel(ctx, tc, x_ap, wq_ap, wk_ap, wv_ap, wp_ap, out_ap)

    nc.compile()
    return nc


def build_kernel(ctx, tc, x_ap, wq_ap, wk_ap, wv_ap, wp_ap, out_ap):
    nc = tc.nc

    # ---------------- constants ----------------
    consts = ctx.enter_context(tc.tile_pool(name="consts", bufs=1))
    ident32 = consts.tile([128, 128], F32)
    make_identity(nc, ident32)
    identb = consts.tile([128, 128], BF16)
    nc.vector.tensor_copy(out=identb, in_=ident32)
    onescol32 = consts.tile([128, HPC], F32)
    nc.vector.memset(onescol32, 1.0)
    # triangle mask tri[k, q] = 1 if q >= k else 0 (used on every diagonal
    # 128x128 block)
    tri32 = consts.tile([128, 128], F32)
    nc.gpsimd.memset(tri32, 1.0)
    nc.gpsimd.affine_select(
        out=tri32,
        in_=tri32,
        compare_op=mybir.AluOpType.is_ge,
        fill=0.0,
        base=0,
        pattern=[[1, 128]],
        channel_multiplier=-1,
    )
    tri = consts.tile([128, 128], BF16)
    nc.vector.tensor_copy(out=tri, in_=tri32)

    # ---------------- persistent tiles ----------------
    persist = ctx.enter_context(tc.tile_pool(name="persist", bufs=1))
    QT = [persist.tile([128, T], BF16, tag=f"qt{i}", name=f"qt{i}") for i in range(4)]
    KT = [persist.tile([128, T], BF16, tag=f"kt{i}", name=f"kt{i}") for i in range(4)]
    V65 = [
        persist.tile([128, HPC * 65], BF16, tag=f"v{i}", name=f"v{i}")
        for i in range(16)
    ]
    for i in range(16):
        nc.scalar.activation(
            out=V65[i].rearrange("p (h e) -> p h e", e=65)[:, :, 64:65],
            in_=onescol32.rearrange("p (h o) -> p h o", o=1),
            func=AF.Copy,
        )
    w_sb = {
        "wq": [], "wk": [], "wv": [],
    }
    for name in ("wq", "wk", "wv"):
        for cb in range(8):
            t = persist.tile(
                [128, GC], BF16, tag=f"{name}{cb}", name=f"{name}{cb}"
            )
            w_sb[name].append(t)
    wp_sb = [
        persist.tile([128, C], BF16, tag=f"wp{cb}", name=f"wpc{cb}")
        for cb in range(4)
    ]

    # yt tiles (normalized attention output, transposed): allocated per panel
    # from a 2-deep ring so proj(Q-1) can read while A(Q) writes.
    ytpool = ctx.enter_context(tc.tile_pool(name="ytp", bufs=2))

    # ---------------- working pools ----------------
    stg_pool = ctx.enter_context(tc.tile_pool(name="stg", bufs=6))
    xb_pool = ctx.enter_context(tc.tile_pool(name="xb", bufs=2))
    xt_pool = ctx.enter_context(tc.tile_pool(name="xt", bufs=2))
    ex_pool = ctx.enter_context(tc.tile_pool(name="ex", bufs=3))
    nrm_pool = ctx.enter_context(tc.tile_pool(name="nrm", bufs=3))
    ot_pool = ctx.enter_context(tc.tile_pool(name="ot", bufs=3))
    ps_sp = ctx.enter_context(tc.tile_pool(name="ps_sp", bufs=4, space="PSUM"))
    ps_y = ctx.enter_context(tc.tile_pool(name="ps_y", bufs=1, space="PSUM"))
    ps_acc = ctx.enter_context(tc.tile_pool(name="ps_acc", bufs=2, space="PSUM"))

    # ---------------- weight load (staged, cast to bf16 on gpsimd) ----------
    def emit_weight_loads():
        for name, ap in (("wq", wq_ap), ("wk", wk_ap), ("wv", wv_ap)):
            for cb in range(8):
                stg = stg_pool.tile([128, C], F32, tag="stg", name="wstg")
                nc.sync.dma_start(
                    out=stg[:, 0:GC], in_=ap[128 * cb : 128 * cb + 128, :]
                )
                nc.gpsimd.tensor_copy(out=w_sb[name][cb], in_=stg[:, 0:GC])

    def emit_wp_load():
        for cb in range(4):
            stg = stg_pool.tile([128, C], F32, tag="stg", name="wpstg")
            nc.sync.dma_start(out=stg, in_=wp_ap[128 * cb : 128 * cb + 128, :])
            nc.gpsimd.tensor_copy(out=wp_sb[cb], in_=stg)

    # ---------------- QKV phase-1 units for one t-panel -------------------
    # state per panel: xb chunks (bf16) and xT tiles
    xbs = {}   # panel -> [4 tiles [128, C] bf16]
    xts = {}   # panel -> [8 tiles [128, 512] bf16]

    def unit_x_dma(p):
        t0 = p * T_PANEL
        chunks = []
        for ts in range(4):
            stg = stg_pool.tile([128, C], F32, tag="stg", name="xstg")
            nc.sync.dma_start(
                out=stg, in_=x_ap[t0 + 128 * ts : t0 + 128 * ts + 128, :]
            )
            chunks.append(stg)
        xbs[p] = [None] * 4
        xbs[(p, "stg")] = chunks

    def unit_x_cast(p, ts):
        xb = xb_pool.tile([128, C], BF16, tag=f"xb{ts}", name=f"xb{ts}")
        nc.vector.tensor_copy(out=xb, in_=xbs[(p, "stg")][ts])
        xbs[p][ts] = xb

    def unit_transpose(p, cb):
        if p not in xts:
            xts[p] = [None] * 8
        xt = xt_pool.tile([128, T_PANEL], BF16, tag=f"xT{cb}", name=f"xT{cb}")
        for ts in range(4):
            pt = ps_acc.tile([128, 128], BF16, tag="acc", name="pt")
            nc.tensor.transpose(
                pt, xbs[p][ts][:, 128 * cb : 128 * cb + 128], identb
            )
            nc.vector.tensor_copy(out=xt[:, 128 * ts : 128 * ts + 128], in_=pt)
        xts[p][cb] = xt

    def unit_qtkt(p, qk, cp):
        dest = QT if qk == "wq" else KT
        t0 = p * T_PANEL
        acc = ps_acc.tile([128, T_PANEL], F32, tag="acc", name="acc")
        for cb in range(8):
            nc.tensor.matmul(
                acc,
                w_sb[qk][cb][:, 128 * cp : 128 * cp + 128],
                xts[p][cb],
                start=(cb == 0),
                stop=(cb == 7),
            )
        nc.vector.tensor_copy(out=dest[cp][:, t0 : t0 + T_PANEL], in_=acc)

    def unit_v(p, ts):
        acc = ps_acc.tile([128, GC], F32, tag="acc", name="acc")
        for cb in range(8):
            nc.tensor.matmul(
                acc,
                xts[p][cb][:, 128 * ts : 128 * ts + 128],
                w_sb["wv"][cb],
                start=(cb == 0),
                stop=(cb == 7),
            )
        vtile = V65[4 * p + ts]
        nc.vector.tensor_copy(
            out=vtile.rearrange("p (h e) -> p h e", e=65)[:, :, 0:64],
            in_=acc.rearrange("p (h e) -> p h e", e=64),
        )

    def panel_units(p):
        units = [lambda: unit_x_dma(p)]
        units += [lambda ts=ts: unit_x_cast(p, ts) for ts in range(4)]
        units += [lambda cb=cb: unit_transpose(p, cb) for cb in range(8)]
        units += [
            lambda qk=qk, cp=cp: unit_qtkt(p, qk, cp)
            for qk in ("wq", "wk")
            for cp in range(4)
        ]
        units += [lambda ts=ts: unit_v(p, ts) for ts in range(4)]
        return units

    # ---------------- output projection units for one q-panel -------------
    def unit_proj(Q, yts, ts):
        q0 = Q * T_PANEL
        ot = ot_pool.tile([128, C], F32, tag="ot", name="ot")
        for co in range(2):
            ops = ps_acc.tile([128, 512], F32, tag="acc", name="ops")
            for cp in range(4):
                nc.tensor.matmul(
                    ops,
                    yts[cp][:, 128 * ts : 128 * ts + 128],
                    wp_sb[cp][:, 512 * co : 512 * co + 512],
                    start=(cp == 0),
                    stop=(cp == 3),
                )
            nc.vector.tensor_copy(out=ot[:, 512 * co : 512 * co + 512], in_=ops)
            nc.sync.dma_start(
                out=out_ap[
                    q0 + 128 * ts : q0 + 128 * ts + 128,
                    512 * co : 512 * co + 512,
                ],
                in_=ot[:, 512 * co : 512 * co + 512],
            )

    def proj_units(Q, yts):
        return [lambda ts=ts: unit_proj(Q, yts, ts) for ts in range(4)]

    # ---------------- attention for one q-panel ---------------------------
    def emit_attention(Q, filler):
        q0 = Q * T_PANEL
        njd = 2 * (Q + 1)
        nslots = 4 * (njd + 1)
        slot = [0]

        def drain():
            slot[0] += 1
            remaining_slots = nslots - slot[0] + 1
            k = -(-len(filler) // remaining_slots) if filler else 0
            for _ in range(k):
                if filler:
                    filler.popleft()()

        yts = [
            ytpool.tile([128, T_PANEL], BF16, tag=f"yt{i}", name=f"yt{i}")
            for i in range(4)
        ]
        for ch in range(4):
            ha, hb = 2 * ch, 2 * ch + 1
            ys = [
                ps_y.tile([128, 512], F32, tag="ya", name="ya"),
                ps_y.tile([128, 512], F32, tag="yb", name="yb"),
            ]
            pending = None

            def emit_pv(jd, exs, last):
                for half in range(2):
                    kb = 2 * jd + half
                    for hi, h in enumerate((ha, hb)):
                        nc.tensor.matmul(
                            ys[hi][0:65, :],
                            V65[kb][:, 65 * h : 65 * h + 65],
                            exs[half][hi],
                            start=(jd == 0 and half == 0),
                            stop=(last and half == 1),
                        )

            for jd in range(njd):
                # QK matmuls (N-trimmed on diagonal blocks), exp emitted
                # eagerly per (half, hi) so the scalar engine starts early
                exs = [[None, None], [None, None]]
                for half in range(2):
                    kb = 2 * jd + half
                    j = kb - 4 * Q
                    lo = 128 * j if j > 0 else 0
                    sps = []
                    for hi in range(2):
                        sp = ps_sp.tile(
                            [128, 512], F32, tag="sp", name=f"sp{hi}{half}"
                        )
                        r0 = 64 * hi
                        nc.tensor.matmul(
                            sp[:, lo:512],
                            KT[ch][r0 : r0 + 64, 128 * kb : 128 * kb + 128],
                            QT[ch][r0 : r0 + 64, q0 + lo : q0 + 512],
                            start=True,
                            stop=True,
                        )
                        sps.append(sp)
                    for hi in range(2):
                        ex = ex_pool.tile(
                            [128, 512], BF16, tag=f"ex{hi}{half}",
                            name=f"ex{hi}{half}",
                        )
                        if lo > 0:
                            nc.gpsimd.memset(ex[:, 0:lo], 0.0)
                        nc.scalar.activation(
                            out=ex[:, lo:512],
                            in_=sps[hi][:, lo:512],
                            func=AF.Exp,
                            scale=SCALE,
                        )
                        if j >= 0:
                            nc.vector.tensor_mul(
                                ex[:, lo : lo + 128], ex[:, lo : lo + 128], tri
                            )
                        exs[half][hi] = ex
                drain()
                if pending is not None:
                    emit_pv(*pending, last=False)
                pending = (jd, exs)
            # extra filler slot so the PE isn't stuck waiting on the last
            # exp before the final PV of the pair
            drain()
            emit_pv(*pending, last=True)

            # normalize: yt rows = ypsum[0:64] * (1/rowsum) broadcast
            for hi, h in enumerate((ha, hb)):
                yu = nrm_pool.tile([64, 512], F32, tag="yu", name="yu")
                nc.vector.tensor_copy(out=yu, in_=ys[hi][0:64, :])
                rs = nrm_pool.tile([1, 512], F32, tag="rs", name="rs")
                nc.vector.tensor_copy(out=rs, in_=ys[hi][64:65, :])
                rec = nrm_pool.tile([1, 512], F32, tag="rec", name="rec")
                nc.vector.reciprocal_approx_fast(out=rec, in_=rs)
                rb = nrm_pool.tile([64, 512], F32, tag="rb", name="rb")
                nc.gpsimd.partition_broadcast(rb, rec)
                r0 = 64 * (h % 2)
                nc.vector.tensor_mul(yts[h // 2][r0 : r0 + 64, :], yu, rb)
        return yts

    # ---------------- top-level schedule ----------------------------------
    # Prologue: x panel 0 + weights, then panel-0 QKV directly.
    unit_x_dma(0)
    emit_weight_loads()
    emit_wp_load()
    for ts in range(4):
        unit_x_cast(0, ts)
    for cb in range(8):
        unit_transpose(0, cb)
    for qk in ("wq", "wk"):
        for cp in range(4):
            unit_qtkt(0, qk, cp)
    for ts in range(4):
        unit_v(0, ts)

    prev_yts = None
    for Q in range(NP):
        filler = deque()
        if Q + 1 < NP:
            filler.extend(panel_units(Q + 1))
        if prev_yts is not None:
            filler.extend(proj_units(Q - 1, prev_yts))
        yts = emit_attention(Q, filler)
        while filler:
            filler.popleft()()
        prev_yts = yts
    for u in proj_units(NP - 1, prev_yts):
        u()


_PROGRAM = None


def _get_program():
    global _PROGRAM
    if _PROGRAM is None:
        _PROGRAM = build_program()
    return _PROGRAM


def make_in_maps(x, w_qkv, w_proj):
    x = np.asarray(x, dtype=np.float32)
    w_qkv = np.asarray(w_qkv, dtype=np.float32)
    w_proj = np.asarray(w_proj, dtype=np.float32)
    in_maps = []
    for core in range(N_CORES):
        b, g = core // 2, core % 2
        c0 = GC * g
        in_maps.append(
            {
                "x": np.ascontiguousarray(x[b]),
                "wq": np.ascontiguousarray(w_qkv[:, c0 : c0 + GC]),
                "wk": np.ascontiguousarray(w_qkv[:, C + c0 : C + c0 + GC]),
                "wv": np.ascontiguousarray(w_qkv[:, 2 * C + c0 : 2 * C + c0 + GC]),
                "wp": np.ascontiguousarray(w_proj[c0 : c0 + GC, :]),
            }
        )
    return in_maps


def combine_outputs(results):
    out = np.empty((B, T, C), dtype=np.float32)
    for b in range(B):
        out[b] = results[2 * b]["out"] + results[2 * b + 1]["out"]
    return out


def kernel(x, w_qkv, w_proj):
    nc = _get_program()
    in_maps = make_in_maps(x, w_qkv, w_proj)
    res = run_bass_kernel_spmd(nc, in_maps, list(range(N_CORES)))
    return combine_outputs(res.results)


if __name__ == "__main__":
    rng = np.random.default_rng(0)
    x = rng.standard_normal((B, T, C), dtype=np.float32)
    wq = rng.standard_normal((C, 3 * C), dtype=np.float32) / 32.0
    wp = rng.standard_normal((C, C), dtype=np.float32) / 32.0
    out = kernel(x, wq, wp)
    print("ok", out.shape, float(np.abs(out).max()))


# revision 17
# speedup vs baseline: 1.0093x; 1.0093x over previous
"""Causal self-attention kernel for 8 trn2 NeuronCores.

Sharding: core c = 2*b + g handles batch b (of 4) and head-group g (of 2,
8 heads each).  Each core computes QKV projection, causal attention and the
partial output projection for its head-group; the host sums the two
head-group partials per batch (the w_proj row-split all-reduce done on host).

All matmuls run in bf16 with fp32 PSUM accumulation, except the x
transposes which run in plain f32 directly on the DMA'd chunks (the bf16
cast is folded into the PSUM-drain copy that follows, so x is never
separately cast).  Causal structure is exploited by
N-trimming the QK matmuls, exp activations and PV matmuls on the diagonal
key blocks; the in-block triangle is masked by an in-place affine_select on
the idle gpsimd engine.  The softmax denominator comes free from a
ones-column appended to V (M=65 PV matmul).
"""

import sys

if "/opt/trn_rl_repo" not in sys.path:
    sys.path.insert(0, "/opt/trn_rl_repo")

from collections import deque
from contextlib import ExitStack

import numpy as np

import concourse.bass as bass
import concourse.mybir as mybir
import concourse.tile as tile
from concourse import bacc
from concourse.bass_utils import run_bass_kernel_spmd
from concourse.masks import make_identity

F32 = mybir.dt.float32
F32R = mybir.dt.float32r
BF16 = mybir.dt.bfloat16
AF = mybir.ActivationFunctionType

B, T, C = 4, 2048, 1024
N_HEAD = 16
HEAD_DIM = 64
N_CORES = 8
HPC = 8          # heads per core
GC = 512         # head-group channel width (8 heads * 64)
SCALE = 0.125    # 1/sqrt(64)

T_PANEL = 512
NP = T // T_PANEL    # 4 panels (shared for t and q)


def build_program():
    nc = bacc.Bacc(
        "TRN2", target_bir_lowering=False, debug=False, num_devices=N_CORES
    )
    x_ap = nc.dram_tensor("x", [T, C], F32, kind="ExternalInput").ap()
    wq_ap = nc.dram_tensor("wq", [C, GC], F32, kind="ExternalInput").ap()
    wk_ap = nc.dram_tensor("wk", [C, GC], F32, kind="ExternalInput").ap()
    wv_ap = nc.dram_tensor("wv", [C, GC], F32, kind="ExternalInput").ap()
    wp_ap = nc.dram_tensor("wp", [GC, C], F32, kind="ExternalInput").ap()
    out_ap = nc.dram_tensor("out", [T, C], F32, kind="ExternalOutput").ap()

    with ExitStack() as ctx:
        tc = ctx.enter_context(tile.TileContext(nc))
        build_kernel(ctx, tc, x_ap, wq_ap, wk_ap, wv_ap, wp_ap, out_ap)

    nc.compile()
    return nc


def build_kernel(ctx, tc, x_ap, wq_ap, wk_ap, wv_ap, wp_ap, out_ap):
    nc = tc.nc

    # ---------------- constants ----------------
    consts = ctx.enter_context(tc.tile_pool(name="consts", bufs=1))
    ident32 = consts.tile([128, 128], F32)
    make_identity(nc, ident32)
    onescol32 = consts.tile([128, HPC], F32)
    nc.vector.memset(onescol32, 1.0)

    # ---------------- persistent tiles ----------------
    persist = ctx.enter_context(tc.tile_pool(name="persist", bufs=1))
    QT = [persist.tile([128, T], BF16, tag=f"qt{i}", name=f"qt{i}") for i in range(4)]
    KT = [persist.tile([128, T], BF16, tag=f"kt{i}", name=f"kt{i}") for i in range(4)]
    V65 = [
        persist.tile([128, HPC * 65], BF16, tag=f"v{i}", name=f"v{i}")
        for i in range(16)
    ]
    for i in range(16):
        nc.scalar.activation(
            out=V65[i].rearrange("p (h e) -> p h e", e=65)[:, :, 64:65],
            in_=onescol32.rearrange("p (h o) -> p h o", o=1),
            func=AF.Copy,
        )
    w_sb = {"wq": [], "wk": [], "wv": []}
    for name in ("wq", "wk", "wv"):
        for cb in range(8):
            w_sb[name].append(
                persist.tile([128, GC], BF16, tag=f"{name}{cb}", name=f"{name}{cb}")
            )
    wp_sb = [
        persist.tile([128, C], BF16, tag=f"wp{cb}", name=f"wpc{cb}")
        for cb in range(4)
    ]

    # yt tiles (normalized attention output, transposed, f32): 2-deep ring so
    # proj(Q-1) can read while A(Q) writes.
    ytpool = ctx.enter_context(tc.tile_pool(name="ytp", bufs=2))

    # ---------------- working pools ----------------
    stg_pool = ctx.enter_context(tc.tile_pool(name="stg", bufs=8))
    xt_pool = ctx.enter_context(tc.tile_pool(name="xt", bufs=2))
    ex_pool = ctx.enter_context(tc.tile_pool(name="ex", bufs=3))
    nrm_pool = ctx.enter_context(tc.tile_pool(name="nrm", bufs=2))
    ot_pool = ctx.enter_context(tc.tile_pool(name="ot", bufs=2))
    ps_sp = ctx.enter_context(tc.tile_pool(name="ps_sp", bufs=4, space="PSUM"))
    ps_y = ctx.enter_context(tc.tile_pool(name="ps_y", bufs=1, space="PSUM"))
    ps_acc = ctx.enter_context(tc.tile_pool(name="ps_acc", bufs=2, space="PSUM"))

    # ---------------- weight loads (staged, cast to bf16 on scalar) --------
    def emit_weight_loads():
        for name, ap in (("wq", wq_ap), ("wk", wk_ap), ("wv", wv_ap)):
            for cb in range(8):
                stg = stg_pool.tile([128, 512], F32, tag="stg", name="wstg")
                nc.sync.dma_start(out=stg, in_=ap[128 * cb : 128 * cb + 128, :])
                nc.scalar.activation(out=w_sb[name][cb], in_=stg, func=AF.Copy)

    def emit_wp_load():
        for cb in range(4):
            for h in range(2):
                stg = stg_pool.tile([128, 512], F32, tag="stg", name="wpstg")
                nc.sync.dma_start(
                    out=stg,
                    in_=wp_ap[128 * cb : 128 * cb + 128, 512 * h : 512 * h + 512],
                )
                nc.scalar.activation(
                    out=wp_sb[cb][:, 512 * h : 512 * h + 512], in_=stg,
                    func=AF.Copy,
                )

    # ---------------- QKV phase-1 units for one t-panel -------------------
    xch = {}   # panel -> dict[(ts, h)] -> [128, 512] f32 chunk
    xts = {}   # panel -> [8 tiles [128, 512] f32]

    def unit_x_dma(p):
        t0 = p * T_PANEL
        xch[p] = {}
        for ts in range(4):
            for h in range(2):
                stg = stg_pool.tile([128, 512], F32, tag="stg", name="xstg")
                nc.sync.dma_start(
                    out=stg,
                    in_=x_ap[
                        t0 + 128 * ts : t0 + 128 * ts + 128,
                        512 * h : 512 * h + 512,
                    ],
                )
                xch[p][(ts, h)] = stg

    def unit_transpose(p, cb):
        if p not in xts:
            xts[p] = [None] * 8
        xt = xt_pool.tile([128, T_PANEL], BF16, tag=f"xT{cb}", name=f"xT{cb}")
        h, c = cb // 4, cb % 4
        for ts in range(4):
            pt = ps_acc.tile([128, 128], F32, tag="acc", name="pt")
            nc.tensor.transpose(
                pt, xch[p][(ts, h)][:, 128 * c : 128 * c + 128], ident32
            )
            nc.vector.tensor_copy(out=xt[:, 128 * ts : 128 * ts + 128], in_=pt)
        xts[p][cb] = xt

    def unit_qtkt(p, qk, cp):
        dest = QT if qk == "wq" else KT
        t0 = p * T_PANEL
        acc = ps_acc.tile([128, T_PANEL], F32, tag="acc", name="acc")
        for cb in range(8):
            nc.tensor.matmul(
                acc,
                w_sb[qk][cb][:, 128 * cp : 128 * cp + 128],
                xts[p][cb],
                start=(cb == 0),
                stop=(cb == 7),
            )
        nc.vector.tensor_copy(out=dest[cp][:, t0 : t0 + T_PANEL], in_=acc)

    def unit_v(p, ts):
        acc = ps_acc.tile([128, GC], F32, tag="acc", name="acc")
        for cb in range(8):
            nc.tensor.matmul(
                acc,
                xts[p][cb][:, 128 * ts : 128 * ts + 128],
                w_sb["wv"][cb],
                start=(cb == 0),
                stop=(cb == 7),
            )
        vtile = V65[4 * p + ts]
        nc.vector.tensor_copy(
            out=vtile.rearrange("p (h e) -> p h e", e=65)[:, :, 0:64],
            in_=acc.rearrange("p (h e) -> p h e", e=64),
        )

    def panel_units(p):
        units = [lambda: unit_x_dma(p)]
        units += [lambda cb=cb: unit_transpose(p, cb) for cb in range(8)]
        units += [
            lambda qk=qk, cp=cp: unit_qtkt(p, qk, cp)
            for qk in ("wq", "wk")
            for cp in range(4)
        ]
        units += [lambda ts=ts: unit_v(p, ts) for ts in range(4)]
        return units

    # ---------------- output projection units for one q-panel -------------
    def unit_proj(Q, yts, ts):
        q0 = Q * T_PANEL
        ot = ot_pool.tile([128, C], F32, tag="ot", name="ot")
        for co in range(2):
            ops = ps_acc.tile([128, 512], F32, tag="acc", name="ops")
            for cp in range(4):
                nc.tensor.matmul(
                    ops,
                    yts[cp][:, 128 * ts : 128 * ts + 128],
                    wp_sb[cp][:, 512 * co : 512 * co + 512],
                    start=(cp == 0),
                    stop=(cp == 3),
                )
            nc.vector.tensor_copy(out=ot[:, 512 * co : 512 * co + 512], in_=ops)
            nc.sync.dma_start(
                out=out_ap[
                    q0 + 128 * ts : q0 + 128 * ts + 128,
                    512 * co : 512 * co + 512,
                ],
                in_=ot[:, 512 * co : 512 * co + 512],
            )

    def proj_units(Q, yts):
        return [lambda ts=ts: unit_proj(Q, yts, ts) for ts in range(4)]

    # ---------------- attention for one q-panel ---------------------------
    def emit_attention(Q, filler):
        q0 = Q * T_PANEL
        njd = 2 * (Q + 1)
        nslots = 4 * (njd + 1)
        slot = [0]

        def drain():
            slot[0] += 1
            remaining_slots = nslots - slot[0] + 1
            k = -(-len(filler) // remaining_slots) if filler else 0
            for _ in range(k):
                if filler:
                    filler.popleft()()

        yts = [
            ytpool.tile([128, T_PANEL], BF16, tag=f"yt{i}", name=f"yt{i}")
            for i in range(4)
        ]
        for ch in range(4):
            ha, hb = 2 * ch, 2 * ch + 1
            ys = [
                ps_y.tile([128, 512], F32, tag="ya", name="ya"),
                ps_y.tile([128, 512], F32, tag="yb", name="yb"),
            ]
            pending = None

            def emit_pv(jd, exs, last):
                for half in range(2):
                    kb = 2 * jd + half
                    j = kb - 4 * Q
                    lo = 128 * j if j > 0 else 0
                    for hi, h in enumerate((ha, hb)):
                        nc.tensor.matmul(
                            ys[hi][0:65, lo:512],
                            V65[kb][:, 65 * h : 65 * h + 65],
                            exs[half][hi][:, lo:512],
                            start=(jd == 0 and half == 0),
                            stop=(last and half == 1),
                            skip_group_check=(lo > 0 or last),
                        )

            for jd in range(njd):
                # QK matmuls (N-trimmed on diagonal blocks), exp emitted
                # eagerly per (half, hi) so the scalar engine starts early
                exs = [[None, None], [None, None]]
                for half in range(2):
                    kb = 2 * jd + half
                    j = kb - 4 * Q
                    lo = 128 * j if j > 0 else 0
                    sps = []
                    for hi in range(2):
                        sp = ps_sp.tile(
                            [128, 512], F32, tag="sp", name=f"sp{hi}{half}"
                        )
                        r0 = 64 * hi
                        nc.tensor.matmul(
                            sp[:, lo:512],
                            KT[ch][r0 : r0 + 64, 128 * kb : 128 * kb + 128],
                            QT[ch][r0 : r0 + 64, q0 + lo : q0 + 512],
                            start=True,
                            stop=True,
                        )
                        sps.append(sp)
                    for hi in range(2):
                        ex = ex_pool.tile(
                            [128, 512], BF16, tag=f"ex{hi}{half}",
                            name=f"ex{hi}{half}",
                        )
                        nc.scalar.activation(
                            out=ex[:, lo:512],
                            in_=sps[hi][:, lo:512],
                            func=AF.Exp,
                            scale=SCALE,
                        )
                        if j >= 0:
                            # zero the upper triangle of the diagonal block
                            nc.gpsimd.affine_select(
                                out=ex[:, lo : lo + 128],
                                in_=ex[:, lo : lo + 128],
                                compare_op=mybir.AluOpType.is_ge,
                                fill=0.0,
                                base=0,
                                pattern=[[1, 128]],
                                channel_multiplier=-1,
                            )
                        exs[half][hi] = ex
                drain()
                if pending is not None:
                    emit_pv(*pending, last=False)
                pending = (jd, exs)
            # extra filler slot so the PE isn't stuck waiting on the last
            # exp before the final PV of the pair
            drain()
            emit_pv(*pending, last=True)

            # normalize: yt rows = ypsum[0:64] * (1/rowsum) broadcast
            for hi, h in enumerate((ha, hb)):
                rs = nrm_pool.tile([1, 512], F32, tag="rs", name="rs")
                nc.vector.tensor_copy(out=rs, in_=ys[hi][64:65, :])
                rec = nrm_pool.tile([1, 512], F32, tag="rec", name="rec")
                nc.vector.reciprocal_approx_fast(out=rec, in_=rs)
                rb = nrm_pool.tile([64, 512], F32, tag="rb", name="rb")
                nc.gpsimd.partition_broadcast(rb, rec)
                r0 = 64 * (h % 2)
                nc.vector.tensor_mul(
                    yts[h // 2][r0 : r0 + 64, :], ys[hi][0:64, :], rb
                )
        return yts

    # ---------------- top-level schedule ----------------------------------
    unit_x_dma(0)
    emit_weight_loads()
    emit_wp_load()
    for cb in range(8):
        unit_transpose(0, cb)
    for qk in ("wq", "wk"):
        for cp in range(4):
            unit_qtkt(0, qk, cp)
    for ts in range(4):
        unit_v(0, ts)

    prev_yts = None
    for Q in range(NP):
        filler = deque()
        if Q + 1 < NP:
            filler.extend(panel_units(Q + 1))
        if prev_yts is not None:
            filler.extend(proj_units(Q - 1, prev_yts))
        yts = emit_attention(Q, filler)
        while filler:
            filler.popleft()()
        prev_yts = yts
    for u in proj_units(NP - 1, prev_yts):
        u()


_PROGRAM = None


def _get_program():
    global _PROGRAM
    if _PROGRAM is None:
        _PROGRAM = build_program()
    return _PROGRAM


def make_in_maps(x, w_qkv, w_proj):
    x = np.asarray(x, dtype=np.float32)
    w_qkv = np.asarray(w_qkv, dtype=np.float32)
    w_proj = np.asarray(w_proj, dtype=np.float32)
    in_maps = []
    for core in range(N_CORES):
        b, g = core // 2, core % 2
        c0 = GC * g
        in_maps.append(
            {
                "x": np.ascontiguousarray(x[b]),
                "wq": np.ascontiguousarray(w_qkv[:, c0 : c0 + GC]),
                "wk": np.ascontiguousarray(w_qkv[:, C + c0 : C + c0 + GC]),
                "wv": np.ascontiguousarray(w_qkv[:, 2 * C + c0 : 2 * C + c0 + GC]),
                "wp": np.ascontiguousarray(w_proj[c0 : c0 + GC, :]),
            }
        )
    return in_maps


def combine_outputs(results):
    out = np.empty((B, T, C), dtype=np.float32)
    for b in range(B):
        out[b] = results[2 * b]["out"] + results[2 * b + 1]["out"]
    return out


def kernel(x, w_qkv, w_proj):
    nc = _get_program()
    in_maps = make_in_maps(x, w_qkv, w_proj)
    res = run_bass_kernel_spmd(nc, in_maps, list(range(N_CORES)))
    return combine_outputs(res.results)


if __name__ == "__main__":
    rng = np.random.default_rng(0)
    x = rng.standard_normal((B, T, C), dtype=np.float32)
    wq = rng.standard_normal((C, 3 * C), dtype=np.float32) / 32.0
    wp = rng.standard_normal((C, C), dtype=np.float32) / 32.0
    out = kernel(x, wq, wp)
    print("ok", out.shape, float(np.abs(out).max()))
